# revision 40
# baseline (speedup 1.0000x reference)
"""AttentionWithContext pooling kernel for Trainium2 (8 NeuronCores, data-parallel).

Computation (per batch row, matching the reference):
    uit = tanh(x @ W + b); ait = uit @ u
    a = exp(ait); a /= (sum(a) + 1e-7); out = x.T @ a

Sharding: pure data parallel over batch. bf16 on-chip compute, fp32 PSUM.

Per-core structure (NB=8 local batches, T=2048, F=256):
  - 32 quarter-batch SWDGE cast-loads (f32 HBM -> bf16 SBUF), all issued upfront
    on a dedicated gpsimd queue (~300 GB/s aggregate, ~56us floor). Dependent
    DMAs are kept off this queue so load descriptor generation never stalls.
  - x transpose for the scores matmul runs on the PE (tensor-engine transpose of
    [128,128] chunks into bf16 PSUM, DVE copies back to SBUF). DMA-xbar
    transposes are avoided for x: each xbar op serializes against all other DMA
    (~6us bubble on this runtime).
  - per quarter (512 T rows): 8 PE transposes + 2 DVE copies -> xTq; PE scores
    W.T @ xTq; ACT tanh(+bias) -> bf16; PE u-dot -> [1,512] PSUM; ACT exp with
    fused partial row-sum.
  - softmax tails: ea rows bounced via internal DRAM (SWDGE); TWO xbar
    transposes total (after batch 3 and batch 7) -> eaT[t%128, c, row];
    PE weighted sum out[1,F] += eaT[:,c,b].T @ x[b][:,c,:]; DVE scale by
    1/(sum+eps); y rows DMA'd at the end.
"""

import sys

for _p in ("/opt/trn_rl_repo",):
    if _p not in sys.path:
        sys.path.append(_p)

import numpy as np

import concourse.bass as bass  # noqa: F401
import concourse.tile as tile
from concourse import bacc, mybir
from concourse.bass_utils import run_bass_kernel_spmd
from concourse.masks import make_identity

B, T, F = 64, 2048, 256
NC = 8
NB = B // NC
TC = T // 128
QT = 4
EPS = 1e-7

BF16 = mybir.dt.bfloat16
F32 = mybir.dt.float32


def _build_tile_kernel(nc):
    x = nc.dram_tensor("x", [NB, T, F], F32, kind="ExternalInput")
    w = nc.dram_tensor("w", [F, F], F32, kind="ExternalInput")
    bb = nc.dram_tensor("b", [F], F32, kind="ExternalInput")
    u = nc.dram_tensor("u", [F], F32, kind="ExternalInput")
    y = nc.dram_tensor("y", [NB, F], F32, kind="ExternalOutput")
    ea_dram = nc.dram_tensor("ea_scratch", [16, T], BF16)

    x_r = x.ap().rearrange("b (c p) f -> b p c f", p=128)

    with tile.TileContext(nc) as tc:
        with (
            tc.tile_pool(name="const", bufs=1) as const,
            tc.tile_pool(name="xpool", bufs=NB) as xpool,
            tc.tile_pool(name="xtpool", bufs=4) as xtpool,
            tc.tile_pool(name="thpool", bufs=4) as thpool,
            tc.tile_pool(name="rowpool", bufs=4) as rowpool,
            tc.tile_pool(name="eatpool", bufs=2) as eatpool,
            tc.tile_pool(name="tps", bufs=2, space="PSUM") as tps,
            tc.tile_pool(name="scps", bufs=3, space="PSUM") as scps,
            tc.tile_pool(name="aitps", bufs=2, space="PSUM") as aitps,
            tc.tile_pool(name="ops", bufs=1, space="PSUM") as ops,
        ):
            w_sb = const.tile([128, 2, F], BF16)
            nc.gpsimd.dma_start(out=w_sb, in_=w.ap().rearrange("(k p) g -> p k g", p=128))
            u_sb = const.tile([128, 2], BF16)
            nc.gpsimd.dma_start(out=u_sb, in_=u.ap().rearrange("(k p) -> p k", p=128))
            b_sb = const.tile([128, 2], F32)
            nc.gpsimd.dma_start(out=b_sb, in_=bb.ap().rearrange("(g p) -> p g", p=128))
            eps_sb = const.tile([1, 1], F32)
            nc.vector.memset(eps_sb, EPS)
            ident = const.tile([128, 128], BF16)
            make_identity(nc, ident)

            # ---- PE warmup during load latency: release the HAM clock gate ----
            warm_ps = ops.tile([128, 4, 128], BF16, tag="o", name="warm_ps")
            for i in range(40):
                nc.tensor.transpose(warm_ps[:, i % 4, :], ident, ident)

            # ---- all x loads upfront (pure gpsimd stream) ----
            xb_tiles = []
            for b in range(NB):
                xb = xpool.tile([128, TC, F], BF16, tag="xb", name=f"xb{b}")
                xb_tiles.append(xb)
                for q in range(QT):
                    cs = slice(q * 4, (q + 1) * 4)
                    nc.gpsimd.dma_start(out=xb[:, cs, :], in_=x_r[b, :, cs, :])

            rinv_tiles = []

            def batch_head(b):
                """scores/tanh/u-dot/exp for one batch."""
                xb = xb_tiles[b]
                th = [thpool.tile([128, T], BF16, tag="th", name=f"th{b}_{i}")
                      for i in range(2)]
                asum = rowpool.tile([1, QT], F32, tag="asum", name=f"asum{b}")
                ea_row = rowpool.tile([1, T], BF16, tag="ea", name=f"ea{b}", bufs=NB)

                for q in range(QT):
                    ts_ = slice(q * 512, (q + 1) * 512)
                    # PE-transpose the quarter: 4 chunks x 2 halves -> psum bf16
                    xTq = []
                    for h in range(2):
                        tp_ps = tps.tile([128, 4, 128], BF16, tag="tp")
                        for j in range(4):
                            c = q * 4 + j
                            nc.tensor.transpose(
                                tp_ps[:, j, :], xb[:, c, h * 128 : (h + 1) * 128], ident
                            )
                        xt = xtpool.tile([128, 4, 128], BF16, tag="xt")
                        nc.vector.tensor_copy(xt, tp_ps)
                        xTq.append(xt)
                    # scores + tanh
                    for gh in range(2):
                        sc = scps.tile([128, 512], F32, tag="sc")
                        for k in range(2):
                            nc.tensor.matmul(
                                sc,
                                lhsT=w_sb[:, k, gh * 128 : (gh + 1) * 128],
                                rhs=xTq[k][:],
                                start=(k == 0),
                                stop=(k == 1),
                            )
                        nc.scalar.activation(
                            out=th[gh][:, ts_], in_=sc,
                            func=mybir.ActivationFunctionType.Tanh,
                            bias=b_sb[:, gh : gh + 1],
                        )

                # u-dot: 4 T-chunks concurrent via PE column-tiling
                ait4 = aitps.tile([128, 512], F32, tag="ait")
                for gh in range(2):
                    for j in range(QT):
                        js = slice(j * 512, (j + 1) * 512)
                        nc.tensor.matmul(
                            ait4[32 * j : 32 * j + 1, :],
                            lhsT=u_sb[:, gh : gh + 1],
                            rhs=th[gh][:, js],
                            start=(gh == 0),
                            stop=(gh == 1),
                            tile_position=(0, 32 * j),
                        )
                for j in range(QT):
                    js = slice(j * 512, (j + 1) * 512)
                    nc.scalar.activation(
                        out=ea_row[:, js], in_=ait4[32 * j : 32 * j + 1, :],
                        func=mybir.ActivationFunctionType.Exp,
                        accum_out=asum[:, j : j + 1],
                    )

                # row -> DRAM bounce (SWDGE; queued after all loads)
                nc.gpsimd.dma_start(out=ea_dram.ap()[b : b + 1, :], in_=ea_row)
                rsum = rowpool.tile([1, 1], F32, tag="rsum")
                nc.vector.reduce_sum(rsum, asum, axis=mybir.AxisListType.X)
                nc.vector.tensor_add(rsum, rsum, eps_sb)
                rinv = rowpool.tile([1, 1], F32, tag="rinv", name=f"rinv{b}", bufs=NB)
                nc.vector.reciprocal(rinv, rsum)
                rinv_tiles.append(rinv)

            y_rows = []

            def wave_tail(bs, eaT):
                """weighted sums for a wave of batches, col-tiled."""
                o4 = ops.tile([128, F], F32, tag="o", name=f"o{bs[0]}")
                for c in range(TC):
                    for j, b in enumerate(bs):
                        nc.tensor.matmul(
                            o4[32 * j : 32 * j + 1, :],
                            lhsT=eaT[:, c, b : b + 1],
                            rhs=xb_tiles[b][:, c, :],
                            start=(c == 0),
                            stop=(c == TC - 1),
                            tile_position=(0, 32 * j),
                        )
                for j, b in enumerate(bs):
                    o_row = rowpool.tile([1, F], F32, tag="orow", name=f"orow{b}", bufs=NB)
                    nc.vector.tensor_scalar_mul(
                        o_row, o4[32 * j : 32 * j + 1, :], rinv_tiles[b]
                    )
                    y_rows.append(o_row)

            for b in range(4):
                batch_head(b)
            eaT_a = eatpool.tile([128, TC, 16], BF16, tag="eaT", name="eaT_a")
            nc.sync.dma_start(out=eaT_a, in_=ea_dram.ap(), transpose=True)
            for b in range(4, NB):
                batch_head(b)
            wave_tail([0, 1, 2, 3], eaT_a)
            eaT_b = eatpool.tile([128, TC, 16], BF16, tag="eaT", name="eaT_b")
            nc.sync.dma_start(out=eaT_b, in_=ea_dram.ap(), transpose=True)
            wave_tail([4, 5, 6, 7], eaT_b)

            for b in range(NB):
                nc.gpsimd.dma_start(out=y.ap()[b : b + 1, :], in_=y_rows[b])

    nc.compile()
    return nc


_NC_CACHE = None


def _get_nc():
    global _NC_CACHE
    if _NC_CACHE is None:
        nc = bacc.Bacc("TRN2", target_bir_lowering=False, debug=False)
        _NC_CACHE = _build_tile_kernel(nc)
    return _NC_CACHE


def _in_maps(x, W, b, u):
    x = np.ascontiguousarray(np.asarray(x, dtype=np.float32))
    W = np.ascontiguousarray(np.asarray(W, dtype=np.float32))
    b = np.ascontiguousarray(np.asarray(b, dtype=np.float32))
    u = np.ascontiguousarray(np.asarray(u, dtype=np.float32))
    return [
        {"x": x[i * NB : (i + 1) * NB], "w": W, "b": b, "u": u} for i in range(NC)
    ]


def kernel(x, W, b, u, _trace=False):
    nc = _get_nc()
    res = run_bass_kernel_spmd(nc, _in_maps(x, W, b, u), core_ids=list(range(NC)),
                               trace=_trace)
    out = np.concatenate([np.asarray(r["y"]) for r in res.results], axis=0)
    if _trace:
        return out, res
    return out


# revision 41
# speedup vs baseline: 1.0098x; 1.0098x over previous
"""AttentionWithContext pooling kernel for Trainium2 (8 NeuronCores, data-parallel).

Computation (per batch row, matching the reference):
    uit = tanh(x @ W + b); ait = uit @ u
    a = exp(ait); a /= (sum(a) + 1e-7); out = x.T @ a

Sharding: pure data parallel over batch. bf16 on-chip compute, fp32 PSUM.

Per-core structure (NB=8 local batches, T=2048, F=256):
  - 32 quarter-batch SWDGE cast-loads (f32 HBM -> bf16 SBUF), all issued upfront
    on a dedicated gpsimd queue (~300 GB/s aggregate, ~56us floor). Dependent
    DMAs are kept off this queue so load descriptor generation never stalls.
  - x transpose for the scores matmul runs on the PE (tensor-engine transpose of
    [128,128] chunks into bf16 PSUM, DVE copies back to SBUF). DMA-xbar
    transposes are avoided for x: each xbar op serializes against all other DMA
    (~6us bubble on this runtime).
  - per quarter (512 T rows): 8 PE transposes + 2 DVE copies -> xTq; PE scores
    W.T @ xTq; ACT tanh(+bias) -> bf16; PE u-dot -> [1,512] PSUM; ACT exp with
    fused partial row-sum.
  - softmax tails: ea rows bounced via internal DRAM (SWDGE); TWO xbar
    transposes total (after batch 3 and batch 7) -> eaT[t%128, c, row];
    PE weighted sum out[1,F] += eaT[:,c,b].T @ x[b][:,c,:]; DVE scale by
    1/(sum+eps); y rows DMA'd at the end.
"""

import sys

for _p in ("/opt/trn_rl_repo",):
    if _p not in sys.path:
        sys.path.append(_p)

import numpy as np

import concourse.bass as bass  # noqa: F401
import concourse.tile as tile
from concourse import bacc, mybir
from concourse.bass_utils import run_bass_kernel_spmd
from concourse.masks import make_identity

B, T, F = 64, 2048, 256
NC = 8
NB = B // NC
TC = T // 128
QT = 4
EPS = 1e-7

BF16 = mybir.dt.bfloat16
F32 = mybir.dt.float32


def _build_tile_kernel(nc):
    x = nc.dram_tensor("x", [NB, T, F], F32, kind="ExternalInput")
    w = nc.dram_tensor("w", [F, F], F32, kind="ExternalInput")
    bb = nc.dram_tensor("b", [F], F32, kind="ExternalInput")
    u = nc.dram_tensor("u", [F], F32, kind="ExternalInput")
    y = nc.dram_tensor("y", [NB, F], F32, kind="ExternalOutput")
    ea_dram = nc.dram_tensor("ea_scratch", [16, T], BF16)

    x_r = x.ap().rearrange("b (c p) f -> b p c f", p=128)

    with tile.TileContext(nc) as tc:
        with (
            tc.tile_pool(name="const", bufs=1) as const,
            tc.tile_pool(name="xpool", bufs=NB) as xpool,
            tc.tile_pool(name="xtpool", bufs=4) as xtpool,
            tc.tile_pool(name="thpool", bufs=4) as thpool,
            tc.tile_pool(name="rowpool", bufs=4) as rowpool,
            tc.tile_pool(name="eatpool", bufs=2) as eatpool,
            tc.tile_pool(name="tps", bufs=2, space="PSUM") as tps,
            tc.tile_pool(name="scps", bufs=3, space="PSUM") as scps,
            tc.tile_pool(name="aitps", bufs=2, space="PSUM") as aitps,
            tc.tile_pool(name="ops", bufs=1, space="PSUM") as ops,
        ):
            w_sb = const.tile([128, 2, F], BF16)
            nc.gpsimd.dma_start(out=w_sb, in_=w.ap().rearrange("(k p) g -> p k g", p=128))
            u_sb = const.tile([128, 2], BF16)
            nc.gpsimd.dma_start(out=u_sb, in_=u.ap().rearrange("(k p) -> p k", p=128))
            b_sb = const.tile([128, 2], F32)
            nc.gpsimd.dma_start(out=b_sb, in_=bb.ap().rearrange("(g p) -> p g", p=128))
            eps_sb = const.tile([1, 1], F32)
            nc.vector.memset(eps_sb, EPS)
            ident = const.tile([128, 128], BF16)
            make_identity(nc, ident)

            # ---- all x loads upfront (pure gpsimd stream) ----
            xb_tiles = []
            for b in range(NB):
                xb = xpool.tile([128, TC, F], BF16, tag="xb", name=f"xb{b}")
                xb_tiles.append(xb)
                for q in range(QT):
                    cs = slice(q * 4, (q + 1) * 4)
                    nc.gpsimd.dma_start(out=xb[:, cs, :], in_=x_r[b, :, cs, :])

            rinv_tiles = []

            def batch_head(b):
                """scores/tanh/u-dot/exp for one batch."""
                xb = xb_tiles[b]
                th = [thpool.tile([128, T], BF16, tag="th", name=f"th{b}_{i}")
                      for i in range(2)]
                asum = rowpool.tile([1, QT], F32, tag="asum", name=f"asum{b}")
                ea_row = rowpool.tile([1, T], BF16, tag="ea", name=f"ea{b}", bufs=NB)

                for q in range(QT):
                    ts_ = slice(q * 512, (q + 1) * 512)
                    # PE-transpose the quarter: 4 chunks x 2 halves -> psum bf16
                    xTq = []
                    for h in range(2):
                        tp_ps = tps.tile([128, 4, 128], BF16, tag="tp")
                        for j in range(4):
                            c = q * 4 + j
                            nc.tensor.transpose(
                                tp_ps[:, j, :], xb[:, c, h * 128 : (h + 1) * 128], ident
                            )
                        xt = xtpool.tile([128, 4, 128], BF16, tag="xt")
                        nc.vector.tensor_copy(xt, tp_ps)
                        xTq.append(xt)
                    # scores + tanh
                    for gh in range(2):
                        sc = scps.tile([128, 512], F32, tag="sc")
                        for k in range(2):
                            nc.tensor.matmul(
                                sc,
                                lhsT=w_sb[:, k, gh * 128 : (gh + 1) * 128],
                                rhs=xTq[k][:],
                                start=(k == 0),
                                stop=(k == 1),
                            )
                        nc.scalar.activation(
                            out=th[gh][:, ts_], in_=sc,
                            func=mybir.ActivationFunctionType.Tanh,
                            bias=b_sb[:, gh : gh + 1],
                        )

                # u-dot: 4 T-chunks concurrent via PE column-tiling
                ait4 = aitps.tile([128, 512], F32, tag="ait")
                for gh in range(2):
                    for j in range(QT):
                        js = slice(j * 512, (j + 1) * 512)
                        nc.tensor.matmul(
                            ait4[32 * j : 32 * j + 1, :],
                            lhsT=u_sb[:, gh : gh + 1],
                            rhs=th[gh][:, js],
                            start=(gh == 0),
                            stop=(gh == 1),
                            tile_position=(0, 32 * j),
                        )
                for j in range(QT):
                    js = slice(j * 512, (j + 1) * 512)
                    nc.scalar.activation(
                        out=ea_row[:, js], in_=ait4[32 * j : 32 * j + 1, :],
                        func=mybir.ActivationFunctionType.Exp,
                        accum_out=asum[:, j : j + 1],
                    )

                # row -> DRAM bounce (SWDGE; queued after all loads)
                nc.gpsimd.dma_start(out=ea_dram.ap()[b : b + 1, :], in_=ea_row)
                rsum = rowpool.tile([1, 1], F32, tag="rsum")
                nc.vector.reduce_sum(rsum, asum, axis=mybir.AxisListType.X)
                nc.vector.tensor_add(rsum, rsum, eps_sb)
                rinv = rowpool.tile([1, 1], F32, tag="rinv", name=f"rinv{b}", bufs=NB)
                nc.vector.reciprocal(rinv, rsum)
                rinv_tiles.append(rinv)

            y_rows = []

            def wave_tail(bs, eaT):
                """weighted sums for a wave of batches, col-tiled."""
                o4 = ops.tile([128, F], F32, tag="o", name=f"o{bs[0]}")
                for c in range(TC):
                    for j, b in enumerate(bs):
                        nc.tensor.matmul(
                            o4[32 * j : 32 * j + 1, :],
                            lhsT=eaT[:, c, b : b + 1],
                            rhs=xb_tiles[b][:, c, :],
                            start=(c == 0),
                            stop=(c == TC - 1),
                            tile_position=(0, 32 * j),
                        )
                for j, b in enumerate(bs):
                    o_row = rowpool.tile([1, F], F32, tag="orow", name=f"orow{b}", bufs=NB)
                    nc.vector.tensor_scalar_mul(
                        o_row, o4[32 * j : 32 * j + 1, :], rinv_tiles[b]
                    )
                    y_rows.append(o_row)

            for b in range(4):
                batch_head(b)
            eaT_a = eatpool.tile([128, TC, 16], BF16, tag="eaT", name="eaT_a")
            nc.sync.dma_start(out=eaT_a, in_=ea_dram.ap(), transpose=True)
            for b in range(4, NB):
                batch_head(b)
            wave_tail([0, 1, 2, 3], eaT_a)
            eaT_b = eatpool.tile([128, TC, 16], BF16, tag="eaT", name="eaT_b")
            nc.sync.dma_start(out=eaT_b, in_=ea_dram.ap(), transpose=True)
            wave_tail([4, 5, 6, 7], eaT_b)

            for b in range(NB):
                nc.gpsimd.dma_start(out=y.ap()[b : b + 1, :], in_=y_rows[b])

    nc.compile()
    return nc


_NC_CACHE = None


def _get_nc():
    global _NC_CACHE
    if _NC_CACHE is None:
        nc = bacc.Bacc("TRN2", target_bir_lowering=False, debug=False)
        _NC_CACHE = _build_tile_kernel(nc)
    return _NC_CACHE


def _in_maps(x, W, b, u):
    x = np.ascontiguousarray(np.asarray(x, dtype=np.float32))
    W = np.ascontiguousarray(np.asarray(W, dtype=np.float32))
    b = np.ascontiguousarray(np.asarray(b, dtype=np.float32))
    u = np.ascontiguousarray(np.asarray(u, dtype=np.float32))
    return [
        {"x": x[i * NB : (i + 1) * NB], "w": W, "b": b, "u": u} for i in range(NC)
    ]


def kernel(x, W, b, u, _trace=False):
    nc = _get_nc()
    res = run_bass_kernel_spmd(nc, _in_maps(x, W, b, u), core_ids=list(range(NC)),
                               trace=_trace)
    out = np.concatenate([np.asarray(r["y"]) for r in res.results], axis=0)
    if _trace:
        return out, res
    return out


# revision 42
# speedup vs baseline: 1.0378x; 1.0278x over previous
"""AttentionWithContext pooling kernel for Trainium2 (8 NeuronCores, data-parallel).

Computation (per batch row, matching the reference):
    uit = tanh(x @ W + b); ait = uit @ u
    a = exp(ait); a /= (sum(a) + 1e-7); out = x.T @ a

Sharding: pure data parallel over batch. bf16 on-chip compute, fp32 PSUM.

Per-core structure (NB=8 local batches, T=2048, F=256):
  - 32 quarter-batch SWDGE cast-loads (f32 HBM -> bf16 SBUF), all issued upfront
    on a dedicated gpsimd queue (~300 GB/s aggregate, ~56us floor). Dependent
    DMAs are kept off this queue so load descriptor generation never stalls.
  - x transpose for the scores matmul runs on the PE (tensor-engine transpose of
    [128,128] chunks into bf16 PSUM, DVE copies back to SBUF). DMA-xbar
    transposes are avoided for x: each xbar op serializes against all other DMA
    (~6us bubble on this runtime).
  - per quarter (512 T rows): 8 PE transposes + 2 DVE copies -> xTq; PE scores
    W.T @ xTq; ACT tanh(+bias) -> bf16; PE u-dot -> [1,512] PSUM; ACT exp with
    fused partial row-sum.
  - softmax tails: ea rows bounced via internal DRAM (SWDGE); TWO xbar
    transposes total (after batch 3 and batch 7) -> eaT[t%128, c, row];
    PE weighted sum out[1,F] += eaT[:,c,b].T @ x[b][:,c,:]; DVE scale by
    1/(sum+eps); y rows DMA'd at the end.
"""

import sys

for _p in ("/opt/trn_rl_repo",):
    if _p not in sys.path:
        sys.path.append(_p)

import numpy as np

import concourse.bass as bass  # noqa: F401
import concourse.tile as tile
from concourse import bacc, mybir
from concourse.bass_utils import run_bass_kernel_spmd
from concourse.masks import make_identity

B, T, F = 64, 2048, 256
NC = 8
NB = B // NC
TC = T // 128
QT = 4
EPS = 1e-7

BF16 = mybir.dt.bfloat16
F32 = mybir.dt.float32


def _build_tile_kernel(nc):
    x = nc.dram_tensor("x", [NB, T, F], F32, kind="ExternalInput")
    w = nc.dram_tensor("w", [F, F], F32, kind="ExternalInput")
    bb = nc.dram_tensor("b", [F], F32, kind="ExternalInput")
    u = nc.dram_tensor("u", [F], F32, kind="ExternalInput")
    y = nc.dram_tensor("y", [NB, F], F32, kind="ExternalOutput")
    ea_dram = nc.dram_tensor("ea_scratch", [16, T], BF16)

    x_r = x.ap().rearrange("b (c p) f -> b p c f", p=128)

    with tile.TileContext(nc) as tc:
        with (
            tc.tile_pool(name="const", bufs=1) as const,
            tc.tile_pool(name="xpool", bufs=NB) as xpool,
            tc.tile_pool(name="xtpool", bufs=4) as xtpool,
            tc.tile_pool(name="thpool", bufs=4) as thpool,
            tc.tile_pool(name="rowpool", bufs=4) as rowpool,
            tc.tile_pool(name="eatpool", bufs=2) as eatpool,
            tc.tile_pool(name="tps", bufs=2, space="PSUM") as tps,
            tc.tile_pool(name="scps", bufs=3, space="PSUM") as scps,
            tc.tile_pool(name="aitps", bufs=2, space="PSUM") as aitps,
            tc.tile_pool(name="ops", bufs=1, space="PSUM") as ops,
        ):
            w_sb = const.tile([128, 2, F], BF16)
            nc.gpsimd.dma_start(out=w_sb, in_=w.ap().rearrange("(k p) g -> p k g", p=128))
            u_sb = const.tile([128, 2], BF16)
            nc.gpsimd.dma_start(out=u_sb, in_=u.ap().rearrange("(k p) -> p k", p=128))
            b_sb = const.tile([128, 2], F32)
            nc.gpsimd.dma_start(out=b_sb, in_=bb.ap().rearrange("(g p) -> p g", p=128))
            eps_sb = const.tile([1, 1], F32)
            nc.vector.memset(eps_sb, EPS)
            ident = const.tile([128, 128], BF16)
            make_identity(nc, ident)

            # ---- all x loads upfront (pure gpsimd stream) ----
            xb_tiles = []
            for b in range(NB):
                xb = xpool.tile([128, TC, F], BF16, tag="xb", name=f"xb{b}")
                xb_tiles.append(xb)
                for q in range(QT):
                    cs = slice(q * 4, (q + 1) * 4)
                    nc.gpsimd.dma_start(out=xb[:, cs, :], in_=x_r[b, :, cs, :])

            rinv_tiles = []

            th_tiles = {}

            def batch_scores(b):
                """transposes/scores/tanh for one batch."""
                xb = xb_tiles[b]
                th = [thpool.tile([128, T], BF16, tag="th", name=f"th{b}_{i}")
                      for i in range(2)]
                th_tiles[b] = th

                for q in range(QT):
                    ts_ = slice(q * 512, (q + 1) * 512)
                    # PE-transpose the quarter: 4 chunks x 2 halves -> psum bf16
                    xTq = []
                    for h in range(2):
                        tp_ps = tps.tile([128, 4, 128], BF16, tag="tp")
                        for j in range(4):
                            c = q * 4 + j
                            nc.tensor.transpose(
                                tp_ps[:, j, :], xb[:, c, h * 128 : (h + 1) * 128], ident
                            )
                        xt = xtpool.tile([128, 4, 128], BF16, tag="xt")
                        nc.vector.tensor_copy(xt, tp_ps)
                        xTq.append(xt)
                    # scores + tanh
                    for gh in range(2):
                        sc = scps.tile([128, 512], F32, tag="sc")
                        for k in range(2):
                            nc.tensor.matmul(
                                sc,
                                lhsT=w_sb[:, k, gh * 128 : (gh + 1) * 128],
                                rhs=xTq[k][:],
                                start=(k == 0),
                                stop=(k == 1),
                            )
                        nc.scalar.activation(
                            out=th[gh][:, ts_], in_=sc,
                            func=mybir.ActivationFunctionType.Tanh,
                            bias=b_sb[:, gh : gh + 1],
                        )

            def batch_softmax(b):
                """u-dot/exp/bounce for one batch (lagged: tanh long done)."""
                th = th_tiles[b]
                asum = rowpool.tile([1, QT], F32, tag="asum", name=f"asum{b}")
                ea_row = rowpool.tile([1, T], BF16, tag="ea", name=f"ea{b}", bufs=NB)
                # u-dot: 4 T-chunks concurrent via PE column-tiling
                ait4 = aitps.tile([128, 512], F32, tag="ait")
                for gh in range(2):
                    for j in range(QT):
                        js = slice(j * 512, (j + 1) * 512)
                        nc.tensor.matmul(
                            ait4[32 * j : 32 * j + 1, :],
                            lhsT=u_sb[:, gh : gh + 1],
                            rhs=th[gh][:, js],
                            start=(gh == 0),
                            stop=(gh == 1),
                            tile_position=(0, 32 * j),
                        )
                for j in range(QT):
                    js = slice(j * 512, (j + 1) * 512)
                    nc.scalar.activation(
                        out=ea_row[:, js], in_=ait4[32 * j : 32 * j + 1, :],
                        func=mybir.ActivationFunctionType.Exp,
                        accum_out=asum[:, j : j + 1],
                    )

                # row -> DRAM bounce (SWDGE; queued after all loads)
                nc.gpsimd.dma_start(out=ea_dram.ap()[b : b + 1, :], in_=ea_row)
                rsum = rowpool.tile([1, 1], F32, tag="rsum")
                nc.vector.reduce_sum(rsum, asum, axis=mybir.AxisListType.X)
                nc.vector.tensor_add(rsum, rsum, eps_sb)
                rinv = rowpool.tile([1, 1], F32, tag="rinv", name=f"rinv{b}", bufs=NB)
                nc.vector.reciprocal(rinv, rsum)
                rinv_tiles.append(rinv)

            y_rows = []

            def wave_tail(bs, eaT):
                """weighted sums for a wave of batches, col-tiled."""
                o4 = ops.tile([128, F], F32, tag="o", name=f"o{bs[0]}")
                for c in range(TC):
                    for j, b in enumerate(bs):
                        nc.tensor.matmul(
                            o4[32 * j : 32 * j + 1, :],
                            lhsT=eaT[:, c, b : b + 1],
                            rhs=xb_tiles[b][:, c, :],
                            start=(c == 0),
                            stop=(c == TC - 1),
                            tile_position=(0, 32 * j),
                        )
                for j, b in enumerate(bs):
                    o_row = rowpool.tile([1, F], F32, tag="orow", name=f"orow{b}", bufs=NB)
                    nc.vector.tensor_scalar_mul(
                        o_row, o4[32 * j : 32 * j + 1, :], rinv_tiles[b]
                    )
                    y_rows.append(o_row)

            eaT_a = None
            for b in range(NB):
                batch_scores(b)
                if b >= 1:
                    batch_softmax(b - 1)
                if b == 4:
                    eaT_a = eatpool.tile([128, TC, 16], BF16, tag="eaT", name="eaT_a")
                    nc.sync.dma_start(out=eaT_a, in_=ea_dram.ap(), transpose=True)
            batch_softmax(NB - 1)
            wave_tail([0, 1, 2, 3], eaT_a)
            eaT_b = eatpool.tile([128, TC, 16], BF16, tag="eaT", name="eaT_b")
            nc.sync.dma_start(out=eaT_b, in_=ea_dram.ap(), transpose=True)
            wave_tail([4, 5, 6, 7], eaT_b)

            for b in range(NB):
                nc.gpsimd.dma_start(out=y.ap()[b : b + 1, :], in_=y_rows[b])

    nc.compile()
    return nc


_NC_CACHE = None


def _get_nc():
    global _NC_CACHE
    if _NC_CACHE is None:
        nc = bacc.Bacc("TRN2", target_bir_lowering=False, debug=False)
        _NC_CACHE = _build_tile_kernel(nc)
    return _NC_CACHE


def _in_maps(x, W, b, u):
    x = np.ascontiguousarray(np.asarray(x, dtype=np.float32))
    W = np.ascontiguousarray(np.asarray(W, dtype=np.float32))
    b = np.ascontiguousarray(np.asarray(b, dtype=np.float32))
    u = np.ascontiguousarray(np.asarray(u, dtype=np.float32))
    return [
        {"x": x[i * NB : (i + 1) * NB], "w": W, "b": b, "u": u} for i in range(NC)
    ]


def kernel(x, W, b, u, _trace=False):
    nc = _get_nc()
    res = run_bass_kernel_spmd(nc, _in_maps(x, W, b, u), core_ids=list(range(NC)),
                               trace=_trace)
    out = np.concatenate([np.asarray(r["y"]) for r in res.results], axis=0)
    if _trace:
        return out, res
    return out


# revision 43
# speedup vs baseline: 1.0447x; 1.0066x over previous
"""AttentionWithContext pooling kernel for Trainium2 (8 NeuronCores, data-parallel).

Computation (per batch row, matching the reference):
    uit = tanh(x @ W + b); ait = uit @ u
    a = exp(ait); a /= (sum(a) + 1e-7); out = x.T @ a

Sharding: pure data parallel over batch. bf16 on-chip compute, fp32 PSUM.

Per-core structure (NB=8 local batches, T=2048, F=256):
  - 32 quarter-batch SWDGE cast-loads (f32 HBM -> bf16 SBUF), all issued upfront
    on a dedicated gpsimd queue (~300 GB/s aggregate, ~56us floor). Dependent
    DMAs are kept off this queue so load descriptor generation never stalls.
  - x transpose for the scores matmul runs on the PE (tensor-engine transpose of
    [128,128] chunks into bf16 PSUM, DVE copies back to SBUF). DMA-xbar
    transposes are avoided for x: each xbar op serializes against all other DMA
    (~6us bubble on this runtime).
  - per quarter (512 T rows): 8 PE transposes + 2 DVE copies -> xTq; PE scores
    W.T @ xTq; ACT tanh(+bias) -> bf16; PE u-dot -> [1,512] PSUM; ACT exp with
    fused partial row-sum.
  - softmax tails: ea rows bounced via internal DRAM (SWDGE); TWO xbar
    transposes total (after batch 3 and batch 7) -> eaT[t%128, c, row];
    PE weighted sum out[1,F] += eaT[:,c,b].T @ x[b][:,c,:]; DVE scale by
    1/(sum+eps); y rows DMA'd at the end.
"""

import sys

for _p in ("/opt/trn_rl_repo",):
    if _p not in sys.path:
        sys.path.append(_p)

import numpy as np

import concourse.bass as bass  # noqa: F401
import concourse.tile as tile
from concourse import bacc, mybir
from concourse.bass_utils import run_bass_kernel_spmd
from concourse.masks import make_identity

B, T, F = 64, 2048, 256
NC = 8
NB = B // NC
TC = T // 128
QT = 4
EPS = 1e-7

BF16 = mybir.dt.bfloat16
F32 = mybir.dt.float32


def _build_tile_kernel(nc):
    x = nc.dram_tensor("x", [NB, T, F], F32, kind="ExternalInput")
    w = nc.dram_tensor("w", [F, F], F32, kind="ExternalInput")
    bb = nc.dram_tensor("b", [F], F32, kind="ExternalInput")
    u = nc.dram_tensor("u", [F], F32, kind="ExternalInput")
    y = nc.dram_tensor("y", [NB, F], F32, kind="ExternalOutput")
    ea_dram = nc.dram_tensor("ea_scratch", [16, T], BF16)

    x_r = x.ap().rearrange("b (c p) f -> b p c f", p=128)

    with tile.TileContext(nc) as tc:
        with (
            tc.tile_pool(name="const", bufs=1) as const,
            tc.tile_pool(name="xpool", bufs=NB) as xpool,
            tc.tile_pool(name="xtpool", bufs=4) as xtpool,
            tc.tile_pool(name="thpool", bufs=4) as thpool,
            tc.tile_pool(name="rowpool", bufs=4) as rowpool,
            tc.tile_pool(name="eatpool", bufs=2) as eatpool,
            tc.tile_pool(name="tps", bufs=2, space="PSUM") as tps,
            tc.tile_pool(name="scps", bufs=3, space="PSUM") as scps,
            tc.tile_pool(name="aitps", bufs=2, space="PSUM") as aitps,
            tc.tile_pool(name="ops", bufs=1, space="PSUM") as ops,
        ):
            w_sb = const.tile([128, 2, F], BF16)
            nc.gpsimd.dma_start(out=w_sb, in_=w.ap().rearrange("(k p) g -> p k g", p=128))
            u_sb = const.tile([128, 2], BF16)
            nc.gpsimd.dma_start(out=u_sb, in_=u.ap().rearrange("(k p) -> p k", p=128))
            b_sb = const.tile([128, 2], F32)
            nc.gpsimd.dma_start(out=b_sb, in_=bb.ap().rearrange("(g p) -> p g", p=128))
            eps_sb = const.tile([1, 1], F32)
            nc.vector.memset(eps_sb, EPS)
            ident = const.tile([128, 128], BF16)
            make_identity(nc, ident)

            # ---- all x loads upfront (pure gpsimd stream) ----
            xb_tiles = []
            for b in range(NB):
                xb = xpool.tile([128, TC, F], BF16, tag="xb", name=f"xb{b}")
                xb_tiles.append(xb)
                for q in range(QT):
                    cs = slice(q * 4, (q + 1) * 4)
                    nc.gpsimd.dma_start(out=xb[:, cs, :], in_=x_r[b, :, cs, :])

            rinv_tiles = []

            def batch_head(b):
                """scores/tanh/u-dot/exp for one batch."""
                xb = xb_tiles[b]
                th = [thpool.tile([128, T], BF16, tag="th", name=f"th{b}_{i}")
                      for i in range(2)]
                asum = rowpool.tile([1, QT], F32, tag="asum", name=f"asum{b}")
                ea_row = rowpool.tile([1, T], BF16, tag="ea", name=f"ea{b}", bufs=NB)

                for q in range(QT):
                    ts_ = slice(q * 512, (q + 1) * 512)
                    # PE-transpose the quarter: 4 chunks x 2 halves -> psum bf16
                    xTq = []
                    for h in range(2):
                        tp_ps = tps.tile([128, 4, 128], BF16, tag="tp")
                        for j in range(4):
                            c = q * 4 + j
                            nc.tensor.transpose(
                                tp_ps[:, j, :], xb[:, c, h * 128 : (h + 1) * 128], ident
                            )
                        xt = xtpool.tile([128, 4, 128], BF16, tag="xt")
                        nc.vector.tensor_copy(xt, tp_ps)
                        xTq.append(xt)
                    # scores + tanh
                    for gh in range(2):
                        sc = scps.tile([128, 512], F32, tag="sc")
                        for k in range(2):
                            nc.tensor.matmul(
                                sc,
                                lhsT=w_sb[:, k, gh * 128 : (gh + 1) * 128],
                                rhs=xTq[k][:],
                                start=(k == 0),
                                stop=(k == 1),
                            )
                        nc.scalar.activation(
                            out=th[gh][:, ts_], in_=sc,
                            func=mybir.ActivationFunctionType.Tanh,
                            bias=b_sb[:, gh : gh + 1],
                        )

                # u-dot: 4 T-chunks concurrent via PE column-tiling
                ait4 = aitps.tile([128, 512], F32, tag="ait")
                for gh in range(2):
                    for j in range(QT):
                        js = slice(j * 512, (j + 1) * 512)
                        nc.tensor.matmul(
                            ait4[32 * j : 32 * j + 1, :],
                            lhsT=u_sb[:, gh : gh + 1],
                            rhs=th[gh][:, js],
                            start=(gh == 0),
                            stop=(gh == 1),
                            tile_position=(0, 32 * j),
                        )
                for j in range(QT):
                    js = slice(j * 512, (j + 1) * 512)
                    nc.scalar.activation(
                        out=ea_row[:, js], in_=ait4[32 * j : 32 * j + 1, :],
                        func=mybir.ActivationFunctionType.Exp,
                        accum_out=asum[:, j : j + 1],
                    )

                # row -> DRAM bounce (SWDGE; queued after all loads)
                nc.gpsimd.dma_start(out=ea_dram.ap()[b : b + 1, :], in_=ea_row)
                rsum = rowpool.tile([1, 1], F32, tag="rsum")
                nc.vector.reduce_sum(rsum, asum, axis=mybir.AxisListType.X)
                nc.vector.tensor_add(rsum, rsum, eps_sb)
                rinv = rowpool.tile([1, 1], F32, tag="rinv", name=f"rinv{b}", bufs=NB)
                nc.vector.reciprocal(rinv, rsum)
                rinv_tiles.append(rinv)

            y_rows = []

            def wave_tail(bs, eaT):
                """weighted sums for a wave of batches, col-tiled."""
                o4 = ops.tile([128, F], F32, tag="o", name=f"o{bs[0]}")
                for c in range(TC):
                    for j, b in enumerate(bs):
                        nc.tensor.matmul(
                            o4[32 * j : 32 * j + 1, :],
                            lhsT=eaT[:, c, b : b + 1],
                            rhs=xb_tiles[b][:, c, :],
                            start=(c == 0),
                            stop=(c == TC - 1),
                            tile_position=(0, 32 * j),
                        )
                for j, b in enumerate(bs):
                    o_row = rowpool.tile([1, F], F32, tag="orow", name=f"orow{b}", bufs=NB)
                    nc.vector.tensor_scalar_mul(
                        o_row, o4[32 * j : 32 * j + 1, :], rinv_tiles[b]
                    )
                    y_rows.append(o_row)

            for b in range(4):
                batch_head(b)
            eaT_a = eatpool.tile([128, TC, 16], BF16, tag="eaT", name="eaT_a")
            nc.sync.dma_start(out=eaT_a, in_=ea_dram.ap(), transpose=True)
            for b in range(4, NB):
                batch_head(b)
            wave_tail([0, 1, 2, 3], eaT_a)
            eaT_b = eatpool.tile([128, TC, 16], BF16, tag="eaT", name="eaT_b")
            nc.sync.dma_start(out=eaT_b, in_=ea_dram.ap(), transpose=True)
            wave_tail([4, 5, 6, 7], eaT_b)

            for b in range(NB):
                nc.gpsimd.dma_start(out=y.ap()[b : b + 1, :], in_=y_rows[b])

    nc.compile()
    return nc


_NC_CACHE = None


def _get_nc():
    global _NC_CACHE
    if _NC_CACHE is None:
        nc = bacc.Bacc("TRN2", target_bir_lowering=False, debug=False)
        _NC_CACHE = _build_tile_kernel(nc)
    return _NC_CACHE


def _in_maps(x, W, b, u):
    x = np.ascontiguousarray(np.asarray(x, dtype=np.float32))
    W = np.ascontiguousarray(np.asarray(W, dtype=np.float32))
    b = np.ascontiguousarray(np.asarray(b, dtype=np.float32))
    u = np.ascontiguousarray(np.asarray(u, dtype=np.float32))
    return [
        {"x": x[i * NB : (i + 1) * NB], "w": W, "b": b, "u": u} for i in range(NC)
    ]


def kernel(x, W, b, u, _trace=False):
    nc = _get_nc()
    res = run_bass_kernel_spmd(nc, _in_maps(x, W, b, u), core_ids=list(range(NC)),
                               trace=_trace)
    out = np.concatenate([np.asarray(r["y"]) for r in res.results], axis=0)
    if _trace:
        return out, res
    return out


# revision 45
# speedup vs baseline: 1.0667x; 1.0210x over previous
"""AttentionWithContext pooling kernel for Trainium2 (8 NeuronCores, data-parallel).

Computation (per batch row, matching the reference):
    uit = tanh(x @ W + b); ait = uit @ u
    a = exp(ait); a /= (sum(a) + 1e-7); out = x.T @ a

Sharding: pure data parallel over batch. bf16 on-chip compute, fp32 PSUM.

Per-core structure (NB=8 local batches, T=2048, F=256):
  - 32 quarter-batch SWDGE cast-loads (f32 HBM -> bf16 SBUF), all issued upfront
    on a dedicated gpsimd queue (~300 GB/s aggregate, ~56us floor). Dependent
    DMAs are kept off this queue so load descriptor generation never stalls.
  - x transpose for the scores matmul runs on the PE (tensor-engine transpose of
    [128,128] chunks into bf16 PSUM, DVE copies back to SBUF). DMA-xbar
    transposes are avoided for x: each xbar op serializes against all other DMA
    (~6us bubble on this runtime).
  - per quarter (512 T rows): 8 PE transposes + 2 DVE copies -> xTq; PE scores
    W.T @ xTq; ACT tanh(+bias) -> bf16; PE u-dot -> [1,512] PSUM; ACT exp with
    fused partial row-sum.
  - softmax tails: ea rows bounced via internal DRAM (SWDGE); TWO xbar
    transposes total (after batch 3 and batch 7) -> eaT[t%128, c, row];
    PE weighted sum out[1,F] += eaT[:,c,b].T @ x[b][:,c,:]; DVE scale by
    1/(sum+eps); y rows DMA'd at the end.
"""

import sys

for _p in ("/opt/trn_rl_repo",):
    if _p not in sys.path:
        sys.path.append(_p)

import numpy as np

import concourse.bass as bass  # noqa: F401
import concourse.tile as tile
from concourse import bacc, mybir
from concourse.bass_utils import run_bass_kernel_spmd
from concourse.masks import make_identity

B, T, F = 64, 2048, 256
NC = 8
NB = B // NC
TC = T // 128
QT = 4
EPS = 1e-7

BF16 = mybir.dt.bfloat16
F32 = mybir.dt.float32


def _build_tile_kernel(nc):
    x = nc.dram_tensor("x", [NB, T, F], F32, kind="ExternalInput")
    w = nc.dram_tensor("w", [F, F], F32, kind="ExternalInput")
    bb = nc.dram_tensor("b", [F], F32, kind="ExternalInput")
    u = nc.dram_tensor("u", [F], F32, kind="ExternalInput")
    y = nc.dram_tensor("y", [NB, F], F32, kind="ExternalOutput")
    ea_dram = nc.dram_tensor("ea_scratch", [16, T], BF16)

    x_r = x.ap().rearrange("b (c p) f -> b p c f", p=128)

    with tile.TileContext(nc) as tc:
        with (
            tc.tile_pool(name="const", bufs=1) as const,
            tc.tile_pool(name="xpool", bufs=NB) as xpool,
            tc.tile_pool(name="xtpool", bufs=4) as xtpool,
            tc.tile_pool(name="thpool", bufs=4) as thpool,
            tc.tile_pool(name="rowpool", bufs=4) as rowpool,
            tc.tile_pool(name="eatpool", bufs=2) as eatpool,
            tc.tile_pool(name="tps", bufs=2, space="PSUM") as tps,
            tc.tile_pool(name="scps", bufs=3, space="PSUM") as scps,
            tc.tile_pool(name="aitps", bufs=2, space="PSUM") as aitps,
            tc.tile_pool(name="ops", bufs=1, space="PSUM") as ops,
        ):
            w_sb = const.tile([128, 2, F], BF16)
            nc.gpsimd.dma_start(out=w_sb, in_=w.ap().rearrange("(k p) g -> p k g", p=128))
            u_sb = const.tile([128, 2], BF16)
            nc.gpsimd.dma_start(out=u_sb, in_=u.ap().rearrange("(k p) -> p k", p=128))
            b_sb = const.tile([128, 2], F32)
            nc.gpsimd.dma_start(out=b_sb, in_=bb.ap().rearrange("(g p) -> p g", p=128))
            eps_sb = const.tile([1, 1], F32)
            nc.vector.memset(eps_sb, EPS)
            ident = const.tile([128, 128], BF16)
            make_identity(nc, ident)

            # ---- all x loads upfront (pure gpsimd stream) ----
            xb_tiles = []
            for b in range(NB):
                xb = xpool.tile([128, TC, F], BF16, tag="xb", name=f"xb{b}")
                xb_tiles.append(xb)
                for q in range(QT):
                    cs = slice(q * 4, (q + 1) * 4)
                    nc.gpsimd.dma_start(out=xb[:, cs, :], in_=x_r[b, :, cs, :])

            rinv_tiles = []

            def batch_head(b):
                """scores/tanh/u-dot/exp for one batch."""
                xb = xb_tiles[b]
                th = [thpool.tile([128, T], BF16, tag="th", name=f"th{b}_{i}")
                      for i in range(2)]
                asum = rowpool.tile([1, QT], F32, tag="asum", name=f"asum{b}")
                ea_row = rowpool.tile([1, T], BF16, tag="ea", name=f"ea{b}", bufs=NB)

                for q in range(QT):
                    ts_ = slice(q * 512, (q + 1) * 512)
                    # PE-transpose the quarter: 4 chunks x 2 halves -> psum bf16
                    xTq = []
                    for h in range(2):
                        tp_ps = tps.tile([128, 4, 128], BF16, tag="tp")
                        for j in range(4):
                            c = q * 4 + j
                            nc.tensor.transpose(
                                tp_ps[:, j, :], xb[:, c, h * 128 : (h + 1) * 128], ident
                            )
                        xt = xtpool.tile([128, 4, 128], BF16, tag="xt")
                        nc.vector.tensor_copy(xt, tp_ps)
                        xTq.append(xt)
                    # scores + tanh
                    for gh in range(2):
                        sc = scps.tile([128, 512], F32, tag="sc")
                        for k in range(2):
                            nc.tensor.matmul(
                                sc,
                                lhsT=w_sb[:, k, gh * 128 : (gh + 1) * 128],
                                rhs=xTq[k][:],
                                start=(k == 0),
                                stop=(k == 1),
                            )
                        nc.scalar.activation(
                            out=th[gh][:, ts_], in_=sc,
                            func=mybir.ActivationFunctionType.Tanh,
                            bias=b_sb[:, gh : gh + 1],
                        )

                # u-dot: 4 T-chunks concurrent via PE column-tiling
                ait4 = aitps.tile([128, 512], F32, tag="ait")
                for gh in range(2):
                    for j in range(QT):
                        js = slice(j * 512, (j + 1) * 512)
                        nc.tensor.matmul(
                            ait4[32 * j : 32 * j + 1, :],
                            lhsT=u_sb[:, gh : gh + 1],
                            rhs=th[gh][:, js],
                            start=(gh == 0),
                            stop=(gh == 1),
                            tile_position=(0, 32 * j),
                        )
                for j in range(QT):
                    js = slice(j * 512, (j + 1) * 512)
                    nc.scalar.activation(
                        out=ea_row[:, js], in_=ait4[32 * j : 32 * j + 1, :],
                        func=mybir.ActivationFunctionType.Exp,
                        accum_out=asum[:, j : j + 1],
                    )

                # row -> DRAM bounce (SWDGE; queued after all loads)
                nc.gpsimd.dma_start(out=ea_dram.ap()[b : b + 1, :], in_=ea_row)
                rsum = rowpool.tile([1, 1], F32, tag="rsum")
                nc.vector.reduce_sum(rsum, asum, axis=mybir.AxisListType.X)
                nc.vector.tensor_add(rsum, rsum, eps_sb)
                rinv = rowpool.tile([1, 1], F32, tag="rinv", name=f"rinv{b}", bufs=NB)
                nc.vector.reciprocal(rinv, rsum)
                rinv_tiles.append(rinv)

            o_all = rowpool.tile([1, NB * F], F32, tag="oall", name="o_all")

            def wave_tail(bs, eaT):
                """weighted sums for a wave of batches, col-tiled."""
                o4 = ops.tile([128, F], F32, tag="o", name=f"o{bs[0]}")
                for c in range(TC):
                    for j, b in enumerate(bs):
                        nc.tensor.matmul(
                            o4[32 * j : 32 * j + 1, :],
                            lhsT=eaT[:, c, b : b + 1],
                            rhs=xb_tiles[b][:, c, :],
                            start=(c == 0),
                            stop=(c == TC - 1),
                            tile_position=(0, 32 * j),
                        )
                for j, b in enumerate(bs):
                    nc.vector.tensor_scalar_mul(
                        o_all[:, b * F : (b + 1) * F],
                        o4[32 * j : 32 * j + 1, :], rinv_tiles[b],
                    )

            for b in range(4):
                batch_head(b)
            eaT_a = eatpool.tile([128, TC, 16], BF16, tag="eaT", name="eaT_a")
            nc.sync.dma_start(out=eaT_a, in_=ea_dram.ap(), transpose=True)
            for b in range(4, NB):
                batch_head(b)
            wave_tail([0, 1, 2, 3], eaT_a)
            eaT_b = eatpool.tile([128, TC, 16], BF16, tag="eaT", name="eaT_b")
            nc.sync.dma_start(out=eaT_b, in_=ea_dram.ap(), transpose=True)
            wave_tail([4, 5, 6, 7], eaT_b)

            nc.gpsimd.dma_start(
                out=y.ap().rearrange("b f -> (b f)"), in_=o_all
            )

    nc.compile()
    return nc


_NC_CACHE = None


def _get_nc():
    global _NC_CACHE
    if _NC_CACHE is None:
        nc = bacc.Bacc("TRN2", target_bir_lowering=False, debug=False)
        _NC_CACHE = _build_tile_kernel(nc)
    return _NC_CACHE


def _in_maps(x, W, b, u):
    x = np.ascontiguousarray(np.asarray(x, dtype=np.float32))
    W = np.ascontiguousarray(np.asarray(W, dtype=np.float32))
    b = np.ascontiguousarray(np.asarray(b, dtype=np.float32))
    u = np.ascontiguousarray(np.asarray(u, dtype=np.float32))
    return [
        {"x": x[i * NB : (i + 1) * NB], "w": W, "b": b, "u": u} for i in range(NC)
    ]


def kernel(x, W, b, u, _trace=False):
    nc = _get_nc()
    res = run_bass_kernel_spmd(nc, _in_maps(x, W, b, u), core_ids=list(range(NC)),
                               trace=_trace)
    out = np.concatenate([np.asarray(r["y"]) for r in res.results], axis=0)
    if _trace:
        return out, res
    return out
